# revision 14
# baseline (speedup 1.0000x reference)
"""Bahdanau-attention alignment kernel for Trainium2 (8 NeuronCores).

Full-input contract: kernel(**inputs) takes the unsharded inputs from
setup_inputs() and returns the full [B, TQ, Y_DIM] output.  Sharding is
data-parallel over the batch dim: batch b runs on core b (B == 8 cores).

Math (per batch b):
    qp[a, q]   = sum_d Wq[a, d] * query[q, d] + Wq_b[a]
    yp[a, t]   = sum_d Wy[a, d] * y[t, d]     + Wy_b[a]
    s[t, q]    = sum_a v[a] * tanh(qp[a, q] + yp[a, t])     (+ v_b, dropped:
                 softmax is shift-invariant)
    att[q, t]  = softmax_t(s[t, q] masked to t < n_wins)
    out[q, d]  = sum_t att[q, t] * y[t, d]

Device mapping highlights:
  * The 20.5M-tanh stream runs on ACT in big free-dim batches; the add
    (qp + yp) runs on DVE tensor_scalar (fp32 2x mode, some offloaded to
    GPSIMD); the weighted reduction over the 128 attention dims runs on PE
    using block-diagonal "v" stationaries so 32 t-rows land at 32-aligned
    PSUM partitions.
  * k-partition layout per score matmul group g: p = a'*32 + t' with
    a = 4g + a', t = 32*tb + t'.
  * qp row-replication tables come from SBUF->SBUF DMA partition-broadcast
    (no engine time); yp bias tables come from one-hot replication matmuls
    + DVE 32x32 stream-transpose, spread across the first superblock.
  * Masked softmax: exp with per-partition additive bias (0 or -1e30) so
    masked lanes underflow to exactly 0; normalization deferred past the
    final bmm by appending a ones-column to y (denominator comes out as
    output column 256).
"""

import numpy as np
import ml_dtypes

B = 8
TQ = 400
TY = 400
QD = 256
YD = 256
A = 128
TPAD = 416  # TY padded to a multiple of 32
NG = 32     # score-matmul groups (4 attention dims each)
CH = 16     # groups per ACT tanh chunk (steady state, ts >= 1)
TB = 13     # 32-wide t blocks (12 full + 1 half)

# tuning knobs
POOL_ADDS_TS0 = 2   # X-adds per ts0 pair-chunk done on GPSIMD (of 8)
POOL_ADDS_CH = 1    # X-adds per steady chunk done on GPSIMD (of 16)

_NC_CACHE = {}


def _build_nc():
    import concourse.bass as bass
    import concourse.tile as tile
    from concourse import bacc, mybir
    from contextlib import ExitStack

    f32 = mybir.dt.float32
    bf16 = mybir.dt.bfloat16
    AF = mybir.ActivationFunctionType

    nc = bacc.Bacc("TRN2", target_bir_lowering=False)

    qT_d = nc.dram_tensor("qT", [2, 128, TQ], f32, kind="ExternalInput")
    yT_d = nc.dram_tensor("yT", [2, 128, TPAD], f32, kind="ExternalInput")
    y_d = nc.dram_tensor("y_r", [TY, YD], f32, kind="ExternalInput")
    wqT_d = nc.dram_tensor("wqT", [2, 128, 128], f32, kind="ExternalInput")
    wyT_d = nc.dram_tensor("wyT", [2, 128, 128], f32, kind="ExternalInput")
    wqb_d = nc.dram_tensor("wqb", [128, 1], f32, kind="ExternalInput")
    wyb_d = nc.dram_tensor("wyb", [128, 1], f32, kind="ExternalInput")
    rep_d = nc.dram_tensor("repbig", [128, 1024], f32, kind="ExternalInput")
    vg_d = nc.dram_tensor("vg", [128, NG * 32], bf16, kind="ExternalInput")
    mask_d = nc.dram_tensor("maskb", [128, 4], f32, kind="ExternalInput")
    out_d = nc.dram_tensor("out", [TQ, YD], f32, kind="ExternalOutput")

    with tile.TileContext(nc) as tc, ExitStack() as ctx:
        const = ctx.enter_context(tc.tile_pool(name="const", bufs=1))
        xpool = ctx.enter_context(tc.tile_pool(name="xpool", bufs=3))
        hpool = ctx.enter_context(tc.tile_pool(name="hpool", bufs=2))
        epool = ctx.enter_context(tc.tile_pool(name="epool", bufs=2))
        bbigp = ctx.enter_context(tc.tile_pool(name="bbigp", bufs=2))
        outp = ctx.enter_context(tc.tile_pool(name="outp", bufs=2))
        spsum = ctx.enter_context(tc.tile_pool(name="spsum", bufs=2, space="PSUM"))
        opsum = ctx.enter_context(tc.tile_pool(name="opsum", bufs=1, space="PSUM"))

        # ---------------- input loads (ordered: critical first) ----------------
        wq_sb = []
        qT_sb = []
        wy_sb = []
        yT_sb = []
        for c in range(2):
            t = const.tile([128, 128], f32, tag=f"wq{c}", name=f"wq_sb{c}")
            nc.sync.dma_start(out=t, in_=wqT_d[c])
            wq_sb.append(t)
        for c in range(2):
            t = const.tile([128, TQ], f32, tag=f"qT{c}", name=f"qT_sb{c}")
            nc.sync.dma_start(out=t, in_=qT_d[c])
            qT_sb.append(t)
        wqb_sb = const.tile([128, 1], f32, tag="wqb")
        nc.sync.dma_start(out=wqb_sb, in_=wqb_d[:, :])
        for c in range(2):
            t = const.tile([128, 128], f32, tag=f"wy{c}", name=f"wy_sb{c}")
            nc.sync.dma_start(out=t, in_=wyT_d[c])
            wy_sb.append(t)
        for c in range(2):
            t = const.tile([128, TPAD], f32, tag=f"yT{c}", name=f"yT_sb{c}")
            nc.sync.dma_start(out=t, in_=yT_d[c])
            yT_sb.append(t)
        wyb_sb = const.tile([128, 1], f32, tag="wyb")
        nc.sync.dma_start(out=wyb_sb, in_=wyb_d[:, :])
        rep_sb = const.tile([128, 1024], f32, tag="rep")
        nc.gpsimd.dma_start(out=rep_sb, in_=rep_d[:, :])
        vg_sb = const.tile([128, NG * 32], bf16, tag="vg")
        nc.gpsimd.dma_start(out=vg_sb, in_=vg_d[:, :])
        mask_sb = const.tile([128, 4], f32, tag="mask")
        nc.gpsimd.dma_start(out=mask_sb, in_=mask_d[:, :])

        yaug_sb = []
        for ts in range(4):
            t_sz = 128 if ts < 3 else TY - 384
            t = const.tile([128, YD + 1], f32, tag=f"yaug{ts}", name=f"yaug_sb{ts}")
            nc.gpsimd.dma_start(
                out=t[0:t_sz, 0:YD], in_=y_d[128 * ts : 128 * ts + t_sz, :]
            )
            nc.gpsimd.memset(t[:, YD : YD + 1], 1.0)
            yaug_sb.append(t)

        # ---------------- base projections ----------------
        # bias folded into the PSUM->SBUF eviction as a tensor_scalar add
        qp_ps = spsum.tile([128, TQ], f32, tag="R", name="qp_ps")
        nc.tensor.matmul(qp_ps, lhsT=wq_sb[0], rhs=qT_sb[0], start=True, stop=False)
        nc.tensor.matmul(qp_ps, lhsT=wq_sb[1], rhs=qT_sb[1], start=False, stop=True)
        qpT_sb = const.tile([128, TQ], f32, tag="qpT")
        nc.vector.tensor_scalar_add(out=qpT_sb, in0=qp_ps, scalar1=wqb_sb[:, 0:1])

        yp_ps = spsum.tile([128, TPAD], f32, tag="R", name="yp_ps")
        nc.tensor.matmul(yp_ps, lhsT=wy_sb[0], rhs=yT_sb[0], start=True, stop=False)
        nc.tensor.matmul(yp_ps, lhsT=wy_sb[1], rhs=yT_sb[1], start=False, stop=True)
        ypT_sb = const.tile([128, TPAD], f32, tag="ypT")
        nc.vector.tensor_scalar_add(out=ypT_sb, in0=yp_ps, scalar1=wyb_sb[:, 0:1])

        # ---------------- per-group prep (emitted just-in-time) ----------------
        # qpr[g][p, q]  = qp[4g + p//32, q]   (via DMA partition-broadcast)
        # bt[g][p, tb]  = yp[4g + p//32, 32*tb + p%32]
        qpr = [None] * NG
        bt = [None] * NG

        def emit_prep(g):
            G, e = g // 8, g % 8

            qt = const.tile([128, TQ], f32, tag=f"qpr{g}", name=f"qpr{g}")
            dma_eng = nc.sync if g % 2 == 0 else nc.gpsimd
            for j in range(4):
                dma_eng.dma_start(
                    out=qt[32 * j : 32 * j + 32, :],
                    in_=qpT_sb[4 * g + j : 4 * g + j + 1, :]
                    .unsqueeze(1)
                    .broadcast_to([1, 32, TQ]),
                )
            qpr[g] = qt

            bp = spsum.tile([128, TPAD], f32, tag="R", name=f"bp{g}")
            nc.tensor.matmul(
                bp,
                lhsT=rep_sb[32 * G : 32 * G + 32, 128 * e : 128 * e + 128],
                rhs=ypT_sb[32 * G : 32 * G + 32, :],
                start=True,
                stop=True,
                tile_position=(32 * G, 0),
            )
            bb = bbigp.tile([128, TPAD], f32, tag="bb", name=f"bb{g}")
            nc.vector.transpose(out=bb, in_=bp)
            btg = const.tile([128, TB], f32, tag=f"bt{g}", name=f"bt{g}")
            nc.gpsimd.tensor_copy(
                out=btg[:, :].unsqueeze(2),
                in_=bb[:, :].rearrange("p (tb c) -> p tb c", c=32)[:, :, 0:1],
            )
            bt[g] = btg

        # ---------------- output accumulators ----------------
        OP = []
        for qb in range(4):
            t = opsum.tile([128, YD + 1], f32, tag=f"O{qb}", name=f"O{qb}")
            OP.append(t)

        # ---------------- main loop ----------------
        # units (tb, g); emit a chunk = one ACT tanh over a list of units.
        def emit_chunk(S_of, units, pool_adds):
            n = len(units)
            key = f"{units[0][0]}_{units[0][1]}"
            X = xpool.tile([128, CH * TQ], f32, tag="X", name=f"X{key}")
            for i, (tb, g) in enumerate(units):
                eng = nc.gpsimd if i < pool_adds else nc.vector
                eng.tensor_scalar_add(
                    out=X[:, i * TQ : (i + 1) * TQ],
                    in0=qpr[g][:, :],
                    scalar1=bt[g][:, tb : tb + 1],
                )
            H = hpool.tile([128, CH * TQ], bf16, tag="H", name=f"H{key}")
            nc.scalar.activation(
                out=H[:, 0 : n * TQ], in_=X[:, 0 : n * TQ], func=AF.Tanh
            )
            for i, (tb, g) in enumerate(units):
                sub = tb % 4
                nc.tensor.matmul(
                    S_of[tb // 4][32 * sub : 32 * sub + 32, :],
                    lhsT=vg_sb[:, 32 * g : 32 * g + 32],
                    rhs=H[:, i * TQ : (i + 1) * TQ],
                    start=(g == 0),
                    stop=(g == NG - 1),
                    tile_position=(0, 32 * sub),
                )

        S_of = {}

        def finish_ts(ts):
            t_sz = 128 if ts < 3 else TY - 384
            S = S_of[ts]
            E = epool.tile([128, TQ], f32, tag="E", name=f"E{ts}")
            nc.scalar.activation(
                out=E[0:t_sz, :],
                in_=S[0:t_sz, :],
                func=AF.Exp,
                bias=mask_sb[0:t_sz, ts : ts + 1],
            )
            for qb in range(4):
                q_sz = 128 if qb < 3 else TQ - 384
                nc.tensor.matmul(
                    OP[qb][0:q_sz, :],
                    lhsT=E[0:t_sz, 128 * qb : 128 * qb + q_sz],
                    rhs=yaug_sb[ts][0:t_sz, :],
                    start=(ts == 0),
                    stop=(ts == 3),
                    skip_group_check=True,
                )

        # ts0: pair-of-groups chunks spanning all 4 t-blocks; prep interleaved.
        # The first two groups go as single-g chunks to start ACT sooner.
        S_of[0] = spsum.tile([128, TQ], f32, tag="S", name="S0")
        for g in range(2):
            emit_prep(g)
            emit_chunk(S_of, [(tb, g) for tb in range(4)], 1)
        for gp in range(1, NG // 2):
            emit_prep(2 * gp)
            emit_prep(2 * gp + 1)
            units = [(tb, 2 * gp + i) for i in range(2) for tb in range(4)]
            emit_chunk(S_of, units, POOL_ADDS_TS0)
        finish_ts(0)

        # ts1..3: steady state, chunks of CH groups for one t-block.
        for ts in range(1, 4):
            nblk = 4 if ts < 3 else 1
            S_of[ts] = spsum.tile([128, TQ], f32, tag="S", name=f"S{ts}")
            for sub in range(nblk):
                tb = 4 * ts + sub
                if tb == 12:
                    # small final chunk: shortens the PE drain before exp(ts3)
                    chunk_plan = [list(range(0, 16)), list(range(16, 28)),
                                  list(range(28, 32))]
                else:
                    chunk_plan = [
                        list(range(CH * c, CH * (c + 1))) for c in range(NG // CH)
                    ]
                for gs in chunk_plan:
                    units = [(tb, g) for g in gs]
                    emit_chunk(S_of, units, POOL_ADDS_CH)
            finish_ts(ts)

        # ---------------- normalize + store ----------------
        for qb in range(4):
            q_sz = 128 if qb < 3 else TQ - 384
            rec = outp.tile([128, 1], f32, tag="rec", name=f"rec{qb}")
            nc.vector.reciprocal(out=rec[0:q_sz, :], in_=OP[qb][0:q_sz, YD : YD + 1])
            osb = outp.tile([128, YD], f32, tag="osb", name=f"osb{qb}")
            nc.vector.tensor_scalar_mul(
                out=osb[0:q_sz, :], in0=OP[qb][0:q_sz, 0:YD], scalar1=rec[0:q_sz, :]
            )
            nc.sync.dma_start(
                out=out_d[128 * qb : 128 * qb + q_sz, :], in_=osb[0:q_sz, :]
            )

    nc.compile()
    return nc


def get_nc():
    if "nc" not in _NC_CACHE:
        _NC_CACHE["nc"] = _build_nc()
    return _NC_CACHE["nc"]


def host_inputs(query, y, Wq_w, Wq_b, Wy_w, Wy_b, v_w, v_b, n_wins_y):
    """Build the per-core in_maps (host-side layout prep only)."""
    query = np.asarray(query, np.float32)
    y = np.asarray(y, np.float32)
    Wq_w = np.asarray(Wq_w, np.float32)
    Wq_b = np.asarray(Wq_b, np.float32)
    Wy_w = np.asarray(Wy_w, np.float32)
    Wy_b = np.asarray(Wy_b, np.float32)
    v = np.asarray(v_w, np.float32).reshape(-1)  # [128]
    n_wins = np.asarray(n_wins_y).astype(np.int64)

    wqT = np.ascontiguousarray(Wq_w.T.reshape(2, 128, 128))  # [c][d][a]
    wyT = np.ascontiguousarray(Wy_w.T.reshape(2, 128, 128))
    wqb = Wq_b.reshape(128, 1).copy()
    wyb = Wy_b.reshape(128, 1).copy()

    # REPBIG[p, 128*e + p'] = 1 iff p % 32 == 4*e + p' // 32
    p = np.arange(128)
    e = np.arange(8)
    pp = np.arange(128)
    rep = (
        (p[:, None, None] % 32) == (4 * e[None, :, None] + pp[None, None, :] // 32)
    ).astype(np.float32)
    repbig = rep.reshape(128, 1024)

    # vg[p, 32*g + m] = v[4g + p//32] if p % 32 == m else 0
    g = np.arange(NG)
    m = np.arange(32)
    vgv = v[(4 * g[None, :, None] + p[:, None, None] // 32)]  # [128, 32, 32] bcast
    vg = np.where((p[:, None, None] % 32) == m[None, None, :], vgv, 0.0)
    vg = vg.reshape(128, NG * 32).astype(ml_dtypes.bfloat16)

    in_maps = []
    for b in range(B):
        maskb = np.where(
            (p[:, None] + 128 * np.arange(4)[None, :]) < int(n_wins[b]), 0.0, -1e30
        ).astype(np.float32)
        yT = np.zeros((2, 128, TPAD), np.float32)
        yT[:, :, :TY] = y[b].T.reshape(2, 128, TY)
        in_maps.append(
            {
                "qT": np.ascontiguousarray(query[b].T.reshape(2, 128, TQ)),
                "yT": yT,
                "y_r": np.ascontiguousarray(y[b]),
                "wqT": wqT,
                "wyT": wyT,
                "wqb": wqb,
                "wyb": wyb,
                "repbig": repbig,
                "vg": vg,
                "maskb": maskb,
            }
        )
    return in_maps


def kernel(**inputs):
    in_maps = host_inputs(**inputs)
    nc = get_nc()
    from concourse.bass_utils import run_bass_kernel_spmd

    res = run_bass_kernel_spmd(nc, in_maps, core_ids=list(range(B)))
    out = np.stack([r["out"] for r in res.results], axis=0)
    return out.astype(np.float32)


# revision 25
# speedup vs baseline: 103.5685x; 103.5685x over previous
"""Bahdanau-attention alignment kernel for Trainium2 (8 NeuronCores).

Full-input contract: kernel(**inputs) takes the unsharded inputs from
setup_inputs() and returns the full [B, TQ, Y_DIM] output.  Sharding is
data-parallel over the batch dim: batch b runs on core b (B == 8 cores).

Math (per batch b):
    qp[a, q]   = sum_d Wq[a, d] * query[q, d] + Wq_b[a]
    yp[a, t]   = sum_d Wy[a, d] * y[t, d]     + Wy_b[a]
    s[t, q]    = sum_a v[a] * tanh(qp[a, q] + yp[a, t])     (+ v_b, dropped:
                 softmax is shift-invariant)
    att[q, t]  = softmax_t(s[t, q] masked to t < n_wins)
    out[q, d]  = sum_t att[q, t] * y[t, d]

Device mapping highlights:
  * The 20.5M-tanh stream runs on ACT in big free-dim batches; the add
    (qp + yp) runs on DVE tensor_scalar (fp32 2x mode, some offloaded to
    GPSIMD); the weighted reduction over the 128 attention dims runs on PE
    using block-diagonal "v" stationaries so 32 t-rows land at 32-aligned
    PSUM partitions.
  * k-partition layout per score matmul group g: p = a'*32 + t' with
    a = 4g + a', t = 32*tb + t'.
  * qp row-replication tables come from SBUF->SBUF DMA partition-broadcast
    (no engine time); yp bias tables come from one-hot replication matmuls
    + DVE 32x32 stream-transpose, spread across the first superblock.
  * Masked softmax: exp with per-partition additive bias (0 or -1e30) so
    masked lanes underflow to exactly 0; normalization deferred past the
    final bmm by appending a ones-column to y (denominator comes out as
    output column 256).
"""

import numpy as np
import ml_dtypes

B = 8
TQ = 400
TY = 400
QD = 256
YD = 256
A = 128
TPAD = 416  # TY padded to a multiple of 32
NG = 32     # score-matmul groups (4 attention dims each)
CH = 16     # groups per ACT tanh chunk (steady state, ts >= 1)
TB = 13     # 32-wide t blocks (12 full + 1 half)

# tuning knobs
POOL_ADDS_TS0 = 2   # X-adds per ts0 pair-chunk done on GPSIMD (of 8)
POOL_ADDS_CH = 1    # X-adds per steady chunk done on GPSIMD (of 16)

_NC_CACHE = {}


def _build_nc():
    import concourse.bass as bass
    import concourse.tile as tile
    from concourse import bacc, mybir
    from contextlib import ExitStack

    f32 = mybir.dt.float32
    bf16 = mybir.dt.bfloat16
    AF = mybir.ActivationFunctionType

    nc = bacc.Bacc("TRN2", target_bir_lowering=False)

    # packed per-projection inputs: [wT chunk0 | wT chunk1 | xT chunk0 | xT chunk1 | bias]
    QP = 2 * 128 + 2 * TQ + 1
    YP = 2 * 128 + 2 * TPAD + 1
    qpack_d = nc.dram_tensor("qpack", [128, QP], f32, kind="ExternalInput")
    ypack_d = nc.dram_tensor("ypack", [128, YP], f32, kind="ExternalInput")
    y_d = nc.dram_tensor("y_r", [TY, YD], f32, kind="ExternalInput")
    rep_d = nc.dram_tensor("repbig", [128, 1024], f32, kind="ExternalInput")
    vg_d = nc.dram_tensor("vg", [128, NG * 32], bf16, kind="ExternalInput")
    mask_d = nc.dram_tensor("maskb", [128, 4], f32, kind="ExternalInput")
    out_d = nc.dram_tensor("out", [TQ, YD], f32, kind="ExternalOutput")

    with tile.TileContext(nc) as tc, ExitStack() as ctx:
        const = ctx.enter_context(tc.tile_pool(name="const", bufs=1))
        xpool = ctx.enter_context(tc.tile_pool(name="xpool", bufs=3))
        hpool = ctx.enter_context(tc.tile_pool(name="hpool", bufs=2))
        epool = ctx.enter_context(tc.tile_pool(name="epool", bufs=2))
        bbigp = ctx.enter_context(tc.tile_pool(name="bbigp", bufs=2))
        outp = ctx.enter_context(tc.tile_pool(name="outp", bufs=2))
        spsum = ctx.enter_context(tc.tile_pool(name="spsum", bufs=2, space="PSUM"))
        opsum = ctx.enter_context(tc.tile_pool(name="opsum", bufs=1, space="PSUM"))

        # ---------------- input loads (ordered: critical first) ----------------
        qpack = const.tile([128, QP], f32, tag="qpack")
        nc.sync.dma_start(out=qpack, in_=qpack_d[:, :])
        wq_sb = [qpack[:, 0:128], qpack[:, 128:256]]
        qT_sb = [qpack[:, 256 : 256 + TQ], qpack[:, 256 + TQ : 256 + 2 * TQ]]
        wqb_sb = qpack[:, QP - 1 : QP]
        ypack = const.tile([128, YP], f32, tag="ypack")
        nc.sync.dma_start(out=ypack, in_=ypack_d[:, :])
        wy_sb = [ypack[:, 0:128], ypack[:, 128:256]]
        yT_sb = [ypack[:, 256 : 256 + TPAD], ypack[:, 256 + TPAD : 256 + 2 * TPAD]]
        wyb_sb = ypack[:, YP - 1 : YP]
        rep_sb = const.tile([128, 1024], f32, tag="rep")
        nc.gpsimd.dma_start(out=rep_sb, in_=rep_d[:, :])
        vg_sb = const.tile([128, NG * 32], bf16, tag="vg")
        nc.gpsimd.dma_start(out=vg_sb, in_=vg_d[:, :])
        mask_sb = const.tile([128, 4], f32, tag="mask")
        yaug_sb = [
            const.tile([128, YD + 1], f32, tag=f"yaug{ts}", name=f"yaug_sb{ts}")
            for ts in range(4)
        ]

        def emit_late_loads():
            # needed only from finish_ts(0) on; emitted after the startup
            # critical path so they don't head-block the gpsimd DMA ring
            nc.gpsimd.dma_start(out=mask_sb, in_=mask_d[:, :])
            for ts in range(4):
                t_sz = 128 if ts < 3 else TY - 384
                t = yaug_sb[ts]
                nc.gpsimd.dma_start(
                    out=t[0:t_sz, 0:YD], in_=y_d[128 * ts : 128 * ts + t_sz, :]
                )
                nc.gpsimd.memset(t[:, YD : YD + 1], 1.0)

        # ---------------- base projections ----------------
        # bias folded into the PSUM->SBUF eviction as a tensor_scalar add
        qp_ps = spsum.tile([128, TQ], f32, tag="R", name="qp_ps")
        nc.tensor.matmul(qp_ps, lhsT=wq_sb[0], rhs=qT_sb[0], start=True, stop=False)
        nc.tensor.matmul(qp_ps, lhsT=wq_sb[1], rhs=qT_sb[1], start=False, stop=True)
        qpT_sb = const.tile([128, TQ], f32, tag="qpT")
        nc.vector.tensor_scalar_add(out=qpT_sb, in0=qp_ps, scalar1=wqb_sb)

        yp_ps = spsum.tile([128, TPAD], f32, tag="R", name="yp_ps")
        nc.tensor.matmul(yp_ps, lhsT=wy_sb[0], rhs=yT_sb[0], start=True, stop=False)
        nc.tensor.matmul(yp_ps, lhsT=wy_sb[1], rhs=yT_sb[1], start=False, stop=True)
        ypT_sb = const.tile([128, TPAD], f32, tag="ypT")
        nc.vector.tensor_scalar_add(out=ypT_sb, in0=yp_ps, scalar1=wyb_sb)

        # ---------------- per-group prep (emitted just-in-time) ----------------
        # qpr[g][p, q]  = qp[4g + p//32, q]   (via DMA partition-broadcast)
        # bt[g][p, tb]  = yp[4g + p//32, 32*tb + p%32]
        qpr = [None] * NG
        bt = [None] * NG

        def emit_prep(g):
            G, e = g // 8, g % 8

            qt = const.tile([128, TQ], f32, tag=f"qpr{g}", name=f"qpr{g}")
            dma_eng = nc.sync if g % 2 == 0 else nc.gpsimd
            for j in range(4):
                dma_eng.dma_start(
                    out=qt[32 * j : 32 * j + 32, :],
                    in_=qpT_sb[4 * g + j : 4 * g + j + 1, :]
                    .unsqueeze(1)
                    .broadcast_to([1, 32, TQ]),
                )
            qpr[g] = qt

            bp = spsum.tile([128, TPAD], f32, tag="R", name=f"bp{g}")
            nc.tensor.matmul(
                bp,
                lhsT=rep_sb[32 * G : 32 * G + 32, 128 * e : 128 * e + 128],
                rhs=ypT_sb[32 * G : 32 * G + 32, :],
                start=True,
                stop=True,
                tile_position=(32 * G, 0),
            )
            bb = bbigp.tile([128, TPAD], f32, tag="bb", name=f"bb{g}")
            nc.vector.transpose(out=bb, in_=bp)
            btg = const.tile([128, TB], f32, tag=f"bt{g}", name=f"bt{g}")
            nc.gpsimd.tensor_copy(
                out=btg[:, :].unsqueeze(2),
                in_=bb[:, :].rearrange("p (tb c) -> p tb c", c=32)[:, :, 0:1],
            )
            bt[g] = btg

        # ---------------- output accumulators ----------------
        OP = []
        for qb in range(4):
            t = opsum.tile([128, YD + 1], f32, tag=f"O{qb}", name=f"O{qb}")
            OP.append(t)

        # ---------------- main loop ----------------
        # units (tb, g); emit a chunk = one ACT tanh over a list of units.
        def emit_chunk(S_of, units, pool_adds):
            n = len(units)
            key = f"{units[0][0]}_{units[0][1]}"
            X = xpool.tile([128, CH * TQ], f32, tag="X", name=f"X{key}")
            for i, (tb, g) in enumerate(units):
                eng = nc.gpsimd if i < pool_adds else nc.vector
                eng.tensor_scalar_add(
                    out=X[:, i * TQ : (i + 1) * TQ],
                    in0=qpr[g][:, :],
                    scalar1=bt[g][:, tb : tb + 1],
                )
            H = hpool.tile([128, CH * TQ], bf16, tag="H", name=f"H{key}")
            nc.scalar.activation(
                out=H[:, 0 : n * TQ], in_=X[:, 0 : n * TQ], func=AF.Tanh
            )
            for i, (tb, g) in enumerate(units):
                sub = tb % 4
                nc.tensor.matmul(
                    S_of[tb // 4][32 * sub : 32 * sub + 32, :],
                    lhsT=vg_sb[:, 32 * g : 32 * g + 32],
                    rhs=H[:, i * TQ : (i + 1) * TQ],
                    start=(g == 0),
                    stop=(g == NG - 1),
                    tile_position=(0, 32 * sub),
                )

        S_of = {}

        def finish_ts(ts):
            t_sz = 128 if ts < 3 else TY - 384
            S = S_of[ts]
            E = epool.tile([128, TQ], f32, tag="E", name=f"E{ts}")
            nc.scalar.activation(
                out=E[0:t_sz, :],
                in_=S[0:t_sz, :],
                func=AF.Exp,
                bias=mask_sb[0:t_sz, ts : ts + 1],
            )
            for qb in range(4):
                q_sz = 128 if qb < 3 else TQ - 384
                nc.tensor.matmul(
                    OP[qb][0:q_sz, :],
                    lhsT=E[0:t_sz, 128 * qb : 128 * qb + q_sz],
                    rhs=yaug_sb[ts][0:t_sz, :],
                    start=(ts == 0),
                    stop=(ts == 3),
                    skip_group_check=True,
                )

        # ts0: pair-of-groups chunks spanning all 4 t-blocks; prep interleaved.
        # The first two groups go as single-g chunks to start ACT sooner.
        S_of[0] = spsum.tile([128, TQ], f32, tag="S", name="S0")
        for g in range(2):
            emit_prep(g)
            emit_chunk(S_of, [(tb, g) for tb in range(4)], 1)
        emit_late_loads()
        for gp in range(1, NG // 2):
            emit_prep(2 * gp)
            emit_prep(2 * gp + 1)
            units = [(tb, 2 * gp + i) for i in range(2) for tb in range(4)]
            emit_chunk(S_of, units, POOL_ADDS_TS0)
        finish_ts(0)

        # ts1..3: steady state, chunks of CH groups for one t-block.
        for ts in range(1, 4):
            nblk = 4 if ts < 3 else 1
            S_of[ts] = spsum.tile([128, TQ], f32, tag="S", name=f"S{ts}")
            for sub in range(nblk):
                tb = 4 * ts + sub
                if tb == 12:
                    # small final chunk: shortens the PE drain before exp(ts3)
                    chunk_plan = [list(range(0, 16)), list(range(16, 28)),
                                  list(range(28, 32))]
                else:
                    chunk_plan = [
                        list(range(CH * c, CH * (c + 1))) for c in range(NG // CH)
                    ]
                for gs in chunk_plan:
                    units = [(tb, g) for g in gs]
                    emit_chunk(S_of, units, POOL_ADDS_CH)
            finish_ts(ts)

        # ---------------- normalize + store ----------------
        for qb in range(4):
            q_sz = 128 if qb < 3 else TQ - 384
            rec = outp.tile([128, 1], f32, tag="rec", name=f"rec{qb}")
            nc.vector.reciprocal(out=rec[0:q_sz, :], in_=OP[qb][0:q_sz, YD : YD + 1])
            osb = outp.tile([128, YD], f32, tag="osb", name=f"osb{qb}")
            nc.vector.tensor_scalar_mul(
                out=osb[0:q_sz, :], in0=OP[qb][0:q_sz, 0:YD], scalar1=rec[0:q_sz, :]
            )
            nc.sync.dma_start(
                out=out_d[128 * qb : 128 * qb + q_sz, :], in_=osb[0:q_sz, :]
            )

    nc.compile()
    return nc


def get_nc():
    if "nc" not in _NC_CACHE:
        _NC_CACHE["nc"] = _build_nc()
    return _NC_CACHE["nc"]


def host_inputs(query, y, Wq_w, Wq_b, Wy_w, Wy_b, v_w, v_b, n_wins_y):
    """Build the per-core in_maps (host-side layout prep only)."""
    query = np.asarray(query, np.float32)
    y = np.asarray(y, np.float32)
    Wq_w = np.asarray(Wq_w, np.float32)
    Wq_b = np.asarray(Wq_b, np.float32)
    Wy_w = np.asarray(Wy_w, np.float32)
    Wy_b = np.asarray(Wy_b, np.float32)
    v = np.asarray(v_w, np.float32).reshape(-1)  # [128]
    n_wins = np.asarray(n_wins_y).astype(np.int64)

    wqT = np.ascontiguousarray(Wq_w.T.reshape(2, 128, 128))  # [c][d][a]
    wyT = np.ascontiguousarray(Wy_w.T.reshape(2, 128, 128))
    wqb = Wq_b.reshape(128, 1).copy()
    wyb = Wy_b.reshape(128, 1).copy()

    # REPBIG[p, 128*e + p'] = 1 iff p % 32 == 4*e + p' // 32
    p = np.arange(128)
    e = np.arange(8)
    pp = np.arange(128)
    rep = (
        (p[:, None, None] % 32) == (4 * e[None, :, None] + pp[None, None, :] // 32)
    ).astype(np.float32)
    repbig = rep.reshape(128, 1024)

    # vg[p, 32*g + m] = v[4g + p//32] if p % 32 == m else 0
    g = np.arange(NG)
    m = np.arange(32)
    vgv = v[(4 * g[None, :, None] + p[:, None, None] // 32)]  # [128, 32, 32] bcast
    vg = np.where((p[:, None, None] % 32) == m[None, None, :], vgv, 0.0)
    vg = vg.reshape(128, NG * 32).astype(ml_dtypes.bfloat16)

    in_maps = []
    for b in range(B):
        maskb = np.where(
            (p[:, None] + 128 * np.arange(4)[None, :]) < int(n_wins[b]), 0.0, -1e30
        ).astype(np.float32)
        yT = np.zeros((2, 128, TPAD), np.float32)
        yT[:, :, :TY] = y[b].T.reshape(2, 128, TY)
        qT = query[b].T.reshape(2, 128, TQ)
        qpack = np.concatenate([wqT[0], wqT[1], qT[0], qT[1], wqb], axis=1)
        ypack = np.concatenate([wyT[0], wyT[1], yT[0], yT[1], wyb], axis=1)
        in_maps.append(
            {
                "qpack": np.ascontiguousarray(qpack, dtype=np.float32),
                "ypack": np.ascontiguousarray(ypack, dtype=np.float32),
                "y_r": np.ascontiguousarray(y[b]),
                "repbig": repbig,
                "vg": vg,
                "maskb": maskb,
            }
        )
    return in_maps


def kernel(**inputs):
    in_maps = host_inputs(**inputs)
    nc = get_nc()
    from concourse.bass_utils import run_bass_kernel_spmd

    res = run_bass_kernel_spmd(nc, in_maps, core_ids=list(range(B)))
    out = np.stack([r["out"] for r in res.results], axis=0)
    return out.astype(np.float32)
